# revision 12
# baseline (speedup 1.0000x reference)
"""Trainium2 Bass kernel for the CazzyAporbo transformer block.

Sharding over 8 NeuronCores: core c handles batch b=c//4 and head group
j=c%4 (4 of 16 heads) for both attention blocks; out-proj partial sums are
combined with an intra-group AllReduce (bf16) after block 1 and a
ReduceScatter (fp32) after block 2.  The ReduceScatter input is laid out
[4, 256, 350] so each core receives exactly its own 350-token strip, on
which it runs the memory-bank read + FFN tail.  The host gathers the 8
[256, 350] strips into the full [2, 1400, 256] output.

All activations are kept feature-major [C, T] on-chip so every matmul
contracts along partitions.  All matmul inputs are bf16 (fp32 accumulate in
PSUM; the residual stream stays fp32 on the vector engine).  Attention
scores are computed transposed ([s, t]) so the softmax denominator comes
out of the attn@v matmul via a ones-column appended to the transposed V.
Local heads live at partition bases {0,32,64,96} of the q/k tiles (matmul
operands must sit at 32-aligned bases).  The affinity bias is folded in as
a host-precomputed exp(0.1*aff).T bf16 multiplier on exp(scores).
"""

import math

import numpy as np

B, T, C = 2, 1400, 256
H, D = 16, 16
FF = 1024
VOCAB = 1400
SLOTS = 256
EPS = 1e-5
P = 128
NC = 8
GROUPS = [[0, 1, 2, 3], [4, 5, 6, 7]]
HL = 4            # heads per core
STRIP = T // 4    # 350
CT = C // P       # 2 C-tiles
T_CHUNKS = [(0, 512), (512, 512), (1024, 376)]
S_CHUNKS = [(i * P, min(P, T - i * P)) for i in range((T + P - 1) // P)]
NSC = len(S_CHUNKS)  # 11

_CACHE = {}


def _import_bass():
    import sys
    for p in ("/opt/trn_rl_repo", "/opt/pypackages"):
        if p not in sys.path:
            sys.path.insert(0, p)
    import ml_dtypes  # noqa: F401
    from concourse import bacc, mybir
    import concourse.bass as bass
    import concourse.tile as tile
    from concourse.bass_utils import run_bass_kernel_spmd
    return bacc, mybir, bass, tile, run_bass_kernel_spmd


def _build_program():
    bacc, mybir, bass, tile, _ = _import_bass()
    dt = mybir.dt
    f32, bf16 = dt.float32, dt.bfloat16
    AF = mybir.ActivationFunctionType
    OP = mybir.AluOpType

    nc = bacc.Bacc("TRN2", target_bir_lowering=False, debug=False, num_devices=NC)

    def din(name, shape, dty=bf16):
        return nc.dram_tensor(name, shape, dty, kind="ExternalInput")

    xT_d = din("xT", [C, T], f32)
    xTb_d = din("xTb", [C, T])
    wq_d = din("wq", [C, P])      # q head h at cols 32h..32h+15 (scaled), rest 0
    wk_d = din("wk", [C, P])
    bq_d = din("bq", [P, 1], f32)
    bk_d = din("bk", [P, 1], f32)
    wv_d = din("wv", [C, HL * D])
    bv_d = din("bv", [HL * D, 1], f32)
    wo_d = din("wo69", [P, C])    # head h rows 32h..: [bias/16, W.T dims, zeros]
    wgq_d = din("wgq", [C, P])
    wgk_d = din("wgk", [C, P])
    bgq_d = din("bgq", [P, 1], f32)
    bgk_d = din("bgk", [P, 1], f32)
    wgv_d = din("wgv", [C, HL * D])
    bgv_d = din("bgv", [HL * D, 1], f32)
    wgo_d = din("wgo69", [P, C])
    af_d = din("expAfT", [T, T])         # exp(0.1*aff).T  [s,t]
    lnw_d = din("lnw", [2, 6, P])        # row0=g, row1=-beta per (ln, ctile)
    lngc_d = din("lngcol", [P, 6], f32)
    ident_d = din("ident", [P, P])
    ind17_d = din("ind17", [HL, P])
    onesr_d = din("onesr", [1, P])
    ones2_d = din("ones2", [P, 2])
    rwT_d = din("readwT", [C, SLOTS])
    rb_d = din("readb", [P, SLOTS // P], f32)
    mb_d = din("membank", [SLOTS, C])
    w1T_d = din("w1T", [C, FF])
    b1_d = din("b1", [P, FF // P], f32)
    w2T_d = din("w2T", [FF, C])
    b2_d = din("b2", [P, CT], f32)
    out_d = nc.dram_tensor("out", [C, STRIP], f32, kind="ExternalOutput")

    with tile.TileContext(nc) as tc:
        with tc.tile_pool(name="const", bufs=1) as cpool, \
             tc.tile_pool(name="act", bufs=1) as apool, \
             tc.tile_pool(name="work", bufs=1) as wpool, \
             tc.tile_pool(name="ps", bufs=4, space="PSUM") as pspool, \
             tc.tile_pool(name="po", bufs=1, space="PSUM") as popool, \
             tc.tile_pool(name="dram", bufs=1, space="DRAM") as dpool:

            def load_const(dram_ap, shape, dty=bf16, tag=None):
                if not isinstance(dram_ap, bass.AP):
                    tag = tag or dram_ap.name
                    dty = dram_ap.dtype
                    dram_ap = dram_ap.ap()
                t = cpool.tile(shape, dty, tag=tag, name=tag)
                nc.sync.dma_start(t[:], dram_ap)
                return t

            # ---------------- constants ----------------
            xT = [load_const(xT_d[ct * P:(ct + 1) * P, :], [P, T], f32, tag=f"xT{ct}")
                  for ct in range(CT)]
            xTb = [load_const(xTb_d[ct * P:(ct + 1) * P, :], [P, T], tag=f"xTb{ct}")
                   for ct in range(CT)]
            wq = [load_const(wq_d[ct * P:(ct + 1) * P, :], [P, P], tag=f"wq{ct}")
                  for ct in range(CT)]
            wk = [load_const(wk_d[ct * P:(ct + 1) * P, :], [P, P], tag=f"wk{ct}")
                  for ct in range(CT)]
            wgq = [load_const(wgq_d[ct * P:(ct + 1) * P, :], [P, P], tag=f"wgq{ct}")
                   for ct in range(CT)]
            wgk = [load_const(wgk_d[ct * P:(ct + 1) * P, :], [P, P], tag=f"wgk{ct}")
                   for ct in range(CT)]
            wv = [load_const(wv_d[ct * P:(ct + 1) * P, :], [P, HL * D], tag=f"wv{ct}")
                  for ct in range(CT)]
            wgv = [load_const(wgv_d[ct * P:(ct + 1) * P, :], [P, HL * D], tag=f"wgv{ct}")
                   for ct in range(CT)]
            bq = load_const(bq_d, [P, 1])
            bk = load_const(bk_d, [P, 1])
            bgq = load_const(bgq_d, [P, 1])
            bgk = load_const(bgk_d, [P, 1])
            bv = load_const(bv_d, [HL * D, 1])
            bgv = load_const(bgv_d, [HL * D, 1])
            wo = load_const(wo_d, [P, C])
            wgo = load_const(wgo_d, [P, C])
            lnw = load_const(lnw_d, [2, 6, P])
            lngc = load_const(lngc_d, [P, 6])
            ident = load_const(ident_d, [P, P])
            ind32 = load_const(ind17_d, [HL, P])
            onesr = load_const(onesr_d, [1, P])
            ones2 = load_const(ones2_d, [P, 2])
            rwT = [load_const(rwT_d[ct * P:(ct + 1) * P, :], [P, SLOTS], tag=f"rwT{ct}")
                   for ct in range(CT)]
            rbias = load_const(rb_d, [P, SLOTS // P])
            mbank = [load_const(mb_d[s * P:(s + 1) * P, :], [P, C], tag=f"mb{s}")
                     for s in range(SLOTS // P)]
            w1T = [load_const(w1T_d[ct * P:(ct + 1) * P, :], [P, FF], tag=f"w1T{ct}")
                   for ct in range(CT)]
            b1 = load_const(b1_d, [P, FF // P])
            w2T = [load_const(w2T_d[k * P:(k + 1) * P, :], [P, C], tag=f"w2T{k}")
                   for k in range(FF // P)]
            b2 = load_const(b2_d, [P, CT])
            ones_col = ones2[:, 0:1]

            # ---------------- layernorm (feature-major) ----------------
            def layer_norm(x_tiles, xb_tiles, ln_idx, n_tok, chunks):
                """h_tiles (bf16) = LN(x) over C; x_tiles f32, xb_tiles bf16."""
                if xb_tiles is None:
                    xb_tiles = [wpool.tile([P, n_tok], bf16, tag=f"xb{ct}",
                                           name=f"xb{ct}") for ct in range(CT)]
                    for ct in range(CT):
                        nc.vector.tensor_copy(xb_tiles[ct][:], x_tiles[ct][:, 0:n_tok])
                sq = [wpool.tile([P, n_tok], bf16, tag=f"sq{ct}", name=f"sq{ct}")
                      for ct in range(CT)]
                for ct in range(CT):
                    nc.scalar.activation(sq[ct][:], x_tiles[ct][:, 0:n_tok], AF.Square)
                A = wpool.tile([1, 3, n_tok], f32, tag="lnA", name="lnA")
                Rr = wpool.tile([1, n_tok], f32, tag="lnR", name="lnR")
                Rb = wpool.tile([1, n_tok], bf16, tag="lnRb", name="lnRb")
                MR = wpool.tile([2, n_tok], bf16, tag="lnMR", name="lnMR")
                nc.vector.memset(MR[:, :], 1.0)
                for (t0, tn) in chunks:
                    psx = pspool.tile([1, tn], f32, tag="big", name="psx")
                    psq = pspool.tile([1, tn], f32, tag="big", name="psq")
                    for ct in range(CT):
                        nc.tensor.matmul(psx, ones_col,
                                         xb_tiles[ct][:, t0:t0 + tn],
                                         start=(ct == 0), stop=(ct == CT - 1))
                    for ct in range(CT):
                        nc.tensor.matmul(psq, ones_col,
                                         sq[ct][:, t0:t0 + tn],
                                         start=(ct == 0), stop=(ct == CT - 1))
                    nc.vector.tensor_copy(A[0:1, 0, t0:t0 + tn], psx[:])
                    nc.vector.tensor_copy(A[0:1, 1, t0:t0 + tn], psq[:])
                # A.,0: sum x -> m ; A.,1: sum x^2 -> var+eps -> sqrt ; A.,2: m^2
                nc.vector.tensor_scalar_mul(A[0:1, 0, :], A[0:1, 0, :], 1.0 / C)
                nc.vector.tensor_tensor(A[0:1, 2, :], A[0:1, 0, :], A[0:1, 0, :],
                                        op=OP.mult)
                nc.vector.tensor_scalar(A[0:1, 1, :], A[0:1, 1, :], 1.0 / C, EPS,
                                        op0=OP.mult, op1=OP.add)
                nc.vector.tensor_tensor(A[0:1, 1, :], A[0:1, 1, :], A[0:1, 2, :],
                                        op=OP.subtract)
                nc.scalar.activation(A[0:1, 1, :], A[0:1, 1, :], AF.Sqrt)
                nc.vector.reciprocal_approx_fast(Rr[0:1, :], A[0:1, 1, :])
                nc.vector.tensor_copy(Rb[0:1, :], Rr[0:1, :])
                nc.vector.tensor_tensor(MR[0:1, :], A[0:1, 0, :], Rr[0:1, :],
                                        op=OP.mult)

                h = [wpool.tile([P, n_tok], bf16, tag=f"h{ct}", name=f"h{ct}")
                     for ct in range(CT)]
                for (t0, tn) in chunks:
                    prb = pspool.tile([P, tn], f32, tag="big", name="prb")
                    nc.tensor.matmul(prb, onesr, Rb[0:1, t0:t0 + tn],
                                     start=True, stop=True)
                    for ct in range(CT):
                        pmg = pspool.tile([P, tn], f32, tag="big", name="pmg")
                        nc.tensor.matmul(pmg, lnw[:, 2 * ln_idx + ct, :],
                                         MR[0:2, t0:t0 + tn], start=True, stop=True)
                        u = wpool.tile([P, tn], f32, tag="lnu", name="lnu")
                        nc.vector.scalar_tensor_tensor(
                            u[:], x_tiles[ct][:, t0:t0 + tn],
                            lngc[:, 2 * ln_idx + ct:2 * ln_idx + ct + 1], prb[:],
                            op0=OP.mult, op1=OP.mult)
                        nc.vector.tensor_tensor(h[ct][:, t0:t0 + tn], u[:], pmg[:],
                                                op=OP.subtract)
                return h

            # ---------------- attention block ----------------
            def attention(h, w_q, b_q, w_k, b_k, w_v, b_v, w_o, use_af, out_cb):
                qT = wpool.tile([P, T], bf16, tag="qT", name="qT")
                kT = wpool.tile([P, T], bf16, tag="kT", name="kT")
                vT = wpool.tile([HL * D, T], bf16, tag="vT", name="vT")
                for (t0, tn) in T_CHUNKS:
                    pq = pspool.tile([P, tn], f32, tag="big", name="pq")
                    pk = pspool.tile([P, tn], f32, tag="big", name="pk")
                    pv = pspool.tile([HL * D, tn], f32, tag="big", name="pv")
                    for ct in range(CT):
                        nc.tensor.matmul(pq, w_q[ct][:], h[ct][:, t0:t0 + tn],
                                         start=(ct == 0), stop=(ct == CT - 1))
                    for ct in range(CT):
                        nc.tensor.matmul(pk, w_k[ct][:], h[ct][:, t0:t0 + tn],
                                         start=(ct == 0), stop=(ct == CT - 1))
                    for ct in range(CT):
                        nc.tensor.matmul(pv, w_v[ct][:], h[ct][:, t0:t0 + tn],
                                         start=(ct == 0), stop=(ct == CT - 1))
                    nc.scalar.activation(qT[:, t0:t0 + tn], pq[:], AF.Identity,
                                         bias=b_q[:])
                    nc.scalar.activation(kT[:, t0:t0 + tn], pk[:], AF.Identity,
                                         bias=b_k[:])
                    nc.scalar.activation(vT[:, t0:t0 + tn], pv[:], AF.Identity,
                                         bias=b_v[:])
                # transpose v -> [s, head, (ones|d|zeros) 32-wide]
                v_s = wpool.tile([P, NSC, HL, 32], bf16, tag="v_s", name="v_s")
                nc.vector.memset(v_s[:, :, :, :], 0.0)
                nc.vector.memset(v_s[:, :, :, 0:1], 1.0)
                for sc, (s0, sn) in enumerate(S_CHUNKS):
                    pt = pspool.tile([P, HL * D], bf16, tag="big", name="pt")
                    nc.tensor.transpose(pt[0:sn, :], vT[:, s0:s0 + sn],
                                        ident[0:HL * D, 0:HL * D])
                    nc.vector.tensor_copy(v_s[0:sn, sc, :, 1:D + 1], pt[0:sn, :])

                for ti, (t0, tn) in enumerate(T_CHUNKS):
                    af_t = None
                    if use_af:
                        af_t = wpool.tile([P, NSC, tn], bf16, tag="af", name="af",
                                          bufs=2)
                        for sc, (s0, sn) in enumerate(S_CHUNKS):
                            nc.sync.dma_start(af_t[0:sn, sc, :],
                                              af_d[s0:s0 + sn, t0:t0 + tn])
                    po = popool.tile([P, tn], f32, tag="po", name="po", bufs=2)
                    for hh in range(HL):
                        E = wpool.tile([P, NSC, tn], bf16, tag="E", name="E", bufs=2)
                        for sc, (s0, sn) in enumerate(S_CHUNKS):
                            pscr = pspool.tile([P, tn], f32, tag="big", name="pscr")
                            nc.tensor.matmul(
                                pscr[0:sn, :],
                                kT[32 * hh:32 * hh + D, s0:s0 + sn],
                                qT[32 * hh:32 * hh + D, t0:t0 + tn],
                                start=True, stop=True,
                                tile_position=(32 * hh, 0))
                            nc.scalar.activation(E[0:sn, sc, :], pscr[0:sn, :], AF.Exp)
                            if use_af:
                                nc.vector.tensor_tensor(E[0:sn, sc, :], E[0:sn, sc, :],
                                                        af_t[0:sn, sc, :], op=OP.mult)
                        for sc, (s0, sn) in enumerate(S_CHUNKS):
                            nc.tensor.matmul(po[32 * hh:32 * hh + 32, :],
                                             v_s[0:sn, sc, hh, :],
                                             E[0:sn, sc, :],
                                             start=(sc == 0), stop=(sc == NSC - 1),
                                             tile_position=(0, 32 * hh))
                    # evac po, gather dens rows {0,32,64,96}, recip, re-broadcast
                    po_sb = wpool.tile([P, tn], f32, tag="po_sb", name="po_sb")
                    nc.vector.tensor_copy(po_sb[:, :], po[:])
                    dens = wpool.tile([HL, tn], f32, tag="dens", name="dens")
                    nc.sync.dma_start(dens[:, :], po_sb[0:P:32, :])
                    rdens = wpool.tile([HL, tn], f32, tag="rdens", name="rdens")
                    nc.vector.reciprocal_approx_fast(rdens[:], dens[:])
                    rdensb = wpool.tile([HL, tn], bf16, tag="rdensb", name="rdensb")
                    nc.vector.tensor_copy(rdensb[:, :], rdens[:, :])
                    prg = pspool.tile([P, tn], f32, tag="big", name="prg")
                    nc.tensor.matmul(prg, ind32[:], rdensb[:], start=True, stop=True)
                    onorm = wpool.tile([P, tn], bf16, tag="onorm", name="onorm")
                    nc.vector.tensor_tensor(onorm[:, :], po_sb[:], prg[:], op=OP.mult)
                    for mc in range(CT):
                        px = pspool.tile([P, tn], f32, tag="big", name="px")
                        nc.tensor.matmul(px, w_o[:, mc * P:(mc + 1) * P],
                                         onorm[:], start=True, stop=True)
                        out_cb(mc, t0, tn, px)

            # ================= block 1: MHA =================
            h1 = layer_norm(xT, xTb, 0, T, T_CHUNKS)
            att1 = [apool.tile([P, T], bf16, tag=f"att1_{ct}", name=f"att1_{ct}")
                    for ct in range(CT)]

            def out1(mc, t0, tn, px):
                nc.vector.tensor_copy(att1[mc][:, t0:t0 + tn], px[:])

            attention(h1, wq, bq, wk, bk, wv, bv, wo, False, out1)

            b1in = dpool.tile([C, T], bf16, tag="b1in", name="b1in")
            b1out = dpool.tile([C, T], bf16, tag="b1out", name="b1out")
            for ct in range(CT):
                nc.sync.dma_start(b1in[ct * P:(ct + 1) * P, :], att1[ct][:])
            nc.gpsimd.collective_compute(
                "AllReduce", mybir.AluOpType.add, replica_groups=GROUPS,
                ins=[b1in.opt()], outs=[b1out.opt()])
            ar1 = [wpool.tile([P, T], bf16, tag=f"ar1_{ct}", name=f"ar1_{ct}")
                   for ct in range(CT)]
            x1 = [apool.tile([P, T], f32, tag=f"x1_{ct}", name=f"x1_{ct}")
                  for ct in range(CT)]
            for ct in range(CT):
                nc.sync.dma_start(ar1[ct][:], b1out[ct * P:(ct + 1) * P, :])
                nc.vector.tensor_tensor(x1[ct][:], xT[ct][:], ar1[ct][:], op=OP.add)

            # ================= block 2: graph attention =================
            h2 = layer_norm(x1, None, 1, T, T_CHUNKS)
            rs2 = [apool.tile([P, T], f32, tag=f"rs2_{ct}", name=f"rs2_{ct}")
                   for ct in range(CT)]

            def out2(mc, t0, tn, px):
                nc.vector.scalar_tensor_tensor(rs2[mc][:, t0:t0 + tn],
                                               x1[mc][:, t0:t0 + tn], 0.25, px[:],
                                               op0=OP.mult, op1=OP.add)

            attention(h2, wgq, bgq, wgk, bgk, wgv, bgv, wgo, True, out2)

            b2in = dpool.tile([4, C, STRIP], f32, tag="b2in", name="b2in")
            b2out = dpool.tile([C, STRIP], f32, tag="b2out", name="b2out")
            for ct in range(CT):
                nc.sync.dma_start(
                    b2in[:, ct * P:(ct + 1) * P, :].rearrange("q p s -> p q s"),
                    rs2[ct][:].rearrange("p (q s) -> p q s", q=4))
            nc.gpsimd.collective_compute(
                "ReduceScatter", mybir.AluOpType.add, replica_groups=GROUPS,
                ins=[b2in.opt()], outs=[b2out.opt()])
            x2 = [wpool.tile([P, STRIP], f32, tag=f"x2_{ct}", name=f"x2_{ct}")
                  for ct in range(CT)]
            x2b = [wpool.tile([P, STRIP], bf16, tag=f"x2b_{ct}", name=f"x2b_{ct}")
                   for ct in range(CT)]
            for ct in range(CT):
                nc.sync.dma_start(x2[ct][:], b2out[ct * P:(ct + 1) * P, :])
                nc.vector.tensor_copy(x2b[ct][:], x2[ct][:])

            # ================= memory-bank read =================
            erw = [wpool.tile([P, STRIP], bf16, tag=f"erw{s}", name=f"erw{s}")
                   for s in range(SLOTS // P)]
            for mc in range(SLOTS // P):
                prw = pspool.tile([P, STRIP], f32, tag="big", name="prw")
                for kc in range(CT):
                    nc.tensor.matmul(prw, rwT[kc][:, mc * P:(mc + 1) * P],
                                     x2b[kc][:], start=(kc == 0),
                                     stop=(kc == CT - 1))
                nc.scalar.activation(erw[mc][:], prw[:], AF.Exp,
                                     bias=rbias[:, mc:mc + 1])
            Sm = wpool.tile([1, STRIP], f32, tag="memstats", name="memstats")
            Smb = wpool.tile([1, STRIP], bf16, tag="memstatsb", name="memstatsb")
            pmd = pspool.tile([1, STRIP], f32, tag="big", name="pmd")
            for s in range(SLOTS // P):
                nc.tensor.matmul(pmd, ones_col, erw[s][:],
                                 start=(s == 0), stop=(s == SLOTS // P - 1))
            nc.vector.reciprocal_approx_fast(Sm[0:1, :], pmd[:])
            nc.vector.tensor_copy(Smb[0:1, :], Sm[0:1, :])
            prc = pspool.tile([P, STRIP], f32, tag="big", name="prc")
            nc.tensor.matmul(prc, onesr, Smb[0:1, :], start=True, stop=True)
            for s in range(SLOTS // P):
                nc.vector.tensor_tensor(erw[s][:], erw[s][:], prc[:], op=OP.mult)
            x3 = [wpool.tile([P, STRIP], f32, tag=f"x3_{ct}", name=f"x3_{ct}")
                  for ct in range(CT)]
            for mc in range(CT):
                pmo = pspool.tile([P, STRIP], f32, tag="big", name="pmo")
                for kc in range(SLOTS // P):
                    nc.tensor.matmul(pmo, mbank[kc][:, mc * P:(mc + 1) * P],
                                     erw[kc][:],
                                     start=(kc == 0), stop=(kc == SLOTS // P - 1))
                nc.vector.tensor_tensor(x3[mc][:], x2[mc][:], pmo[:], op=OP.add)

            # ================= FFN =================
            h3 = layer_norm(x3, None, 2, STRIP, [(0, STRIP)])
            gs = wpool.tile([P, FF // P, STRIP], bf16, tag="gs", name="gs")
            for mc in range(FF // P):
                pf = pspool.tile([P, STRIP], f32, tag="big", name="pf")
                for kc in range(CT):
                    nc.tensor.matmul(pf, w1T[kc][:, mc * P:(mc + 1) * P],
                                     h3[kc][:], start=(kc == 0), stop=(kc == CT - 1))
                nc.scalar.activation(gs[:, mc, :], pf[:], AF.Gelu,
                                     bias=b1[:, mc:mc + 1])
            outT = [wpool.tile([P, STRIP], f32, tag=f"outT{ct}", name=f"outT{ct}")
                    for ct in range(CT)]
            for mc in range(CT):
                pg = popool.tile([P, STRIP], f32, tag="po0", name="pg")
                for kc in range(FF // P):
                    nc.tensor.matmul(pg, w2T[kc][:, mc * P:(mc + 1) * P],
                                     gs[:, kc, :], start=(kc == 0),
                                     stop=(kc == FF // P - 1))
                nc.vector.scalar_tensor_tensor(outT[mc][:], pg[:],
                                               b2[:, mc:mc + 1], x3[mc][:],
                                               op0=OP.add, op1=OP.add)
                nc.sync.dma_start(out_d[mc * P:(mc + 1) * P, :], outT[mc][:])

    nc.compile()
    return nc


def _pad_qk(w, b, rows, scale, bf):
    """[C,128] bf16 padded weights + [128,1] f32 bias: head h at cols 32h+.."""
    wp = np.zeros((C, P), np.float32)
    bp = np.zeros((P, 1), np.float32)
    for hh in range(HL):
        r = rows[hh]
        wp[:, 32 * hh:32 * hh + D] = w[r].T * scale
        bp[32 * hh:32 * hh + D, 0] = b[r] * scale
    return wp.astype(bf), bp


def _host_prep(inputs):
    import ml_dtypes
    bfd = ml_dtypes.bfloat16
    f32 = np.float32
    x = np.asarray(inputs["x"], f32)
    ids = np.asarray(inputs["disease_ids"]).astype(np.int64)
    scale = 1.0 / math.sqrt(D)

    def t32(a):
        return np.ascontiguousarray(np.asarray(a, f32))

    def tb(a):
        return np.ascontiguousarray(np.asarray(a, f32).astype(bfd))

    in_w = t32(inputs["in_proj_w"]); in_b = t32(inputs["in_proj_b"])
    out_w = t32(inputs["out_proj_w"]); out_b = t32(inputs["out_proj_b"])
    gq_w = t32(inputs["gq_w"]); gq_b = t32(inputs["gq_b"])
    gk_w = t32(inputs["gk_w"]); gk_b = t32(inputs["gk_b"])
    gv_w = t32(inputs["gv_w"]); gv_b = t32(inputs["gv_b"])
    go_w = t32(inputs["go_w"]); go_b = t32(inputs["go_b"])
    aff = t32(inputs["affinity"])
    mem = t32(inputs["mem_bank"]); rw = t32(inputs["read_w"]); rb = t32(inputs["read_b"])
    w1 = t32(inputs["ffn_w1"]); b1 = t32(inputs["ffn_b1"])
    w2 = t32(inputs["ffn_w2"]); b2 = t32(inputs["ffn_b2"])

    lnw = np.zeros((2, 6, P), f32)
    lngc = np.zeros((P, 6), f32)
    for i, (g, b) in enumerate([(inputs["ln1_g"], inputs["ln1_b"]),
                                (inputs["ln2_g"], inputs["ln2_b"]),
                                (inputs["ln3_g"], inputs["ln3_b"])]):
        g = t32(g); b = t32(b)
        for ct in range(CT):
            lnw[0, 2 * i + ct] = g[ct * P:(ct + 1) * P]
            lnw[1, 2 * i + ct] = -b[ct * P:(ct + 1) * P]
            lngc[:, 2 * i + ct] = g[ct * P:(ct + 1) * P]

    ind32 = np.zeros((HL, P), f32)
    for hh in range(HL):
        ind32[hh, 32 * hh:32 * hh + 32] = 1.0

    common = dict(
        lnw=tb(lnw), lngcol=lngc, ident=tb(np.eye(P)), onesr=tb(np.ones((1, P))),
        ones2=tb(np.ones((P, 2))), ind17=tb(ind32),
        readwT=tb(rw.T), readb=np.ascontiguousarray(rb.reshape(SLOTS // P, P).T),
        membank=tb(mem), w1T=tb(w1.T),
        b1=np.ascontiguousarray(b1.reshape(FF // P, P).T),
        w2T=tb(w2.T), b2=np.ascontiguousarray(b2.reshape(CT, P).T),
    )

    def build_wo69(w_o, b_o, h0):
        wo69 = np.zeros((P, C), np.float32)
        for hh in range(HL):
            cols = slice((h0 + hh) * D, (h0 + hh + 1) * D)
            wo69[32 * hh, :] = b_o / 16.0
            wo69[32 * hh + 1:32 * hh + 1 + D, :] = w_o[:, cols].T
        return wo69.astype(bfd)

    expAfT = {}
    for b in range(B):
        a = np.exp(0.1 * aff[ids[b]]).T  # [s, t]
        expAfT[b] = np.ascontiguousarray(a.astype(bfd))

    in_maps = []
    for c in range(NC):
        b = c // 4
        h0 = (c % 4) * HL
        q_rows = [slice((h0 + hh) * D, (h0 + hh + 1) * D) for hh in range(HL)]
        k_rows = [slice(C + (h0 + hh) * D, C + (h0 + hh + 1) * D) for hh in range(HL)]
        v_rows = slice(2 * C + h0 * D, 2 * C + (h0 + HL) * D)
        wq_p, bq_p = _pad_qk(in_w, in_b, q_rows, scale, bfd)
        wk_p, bk_p = _pad_qk(in_w, in_b, k_rows, 1.0, bfd)
        g_rows = [slice((h0 + hh) * D, (h0 + hh + 1) * D) for hh in range(HL)]
        wgq_p, bgq_p = _pad_qk(gq_w, gq_b, g_rows, scale, bfd)
        wgk_p, bgk_p = _pad_qk(gk_w, gk_b, g_rows, 1.0, bfd)
        m = dict(common)
        m.update(
            xT=np.ascontiguousarray(x[b].T),
            xTb=tb(x[b].T),
            wq=wq_p, bq=bq_p, wk=wk_p, bk=bk_p,
            wv=tb(in_w[v_rows].T), bv=in_b[v_rows][:, None].copy(),
            wo69=build_wo69(out_w, out_b, h0),
            wgq=wgq_p, bgq=bgq_p, wgk=wgk_p, bgk=bgk_p,
            wgv=tb(gv_w[h0 * D:(h0 + HL) * D].T),
            bgv=gv_b[h0 * D:(h0 + HL) * D][:, None].copy(),
            wgo69=build_wo69(go_w, go_b, h0),
            expAfT=expAfT[b],
        )
        in_maps.append(m)
    return in_maps


def kernel(**inputs):
    _, _, _, _, run_bass_kernel_spmd = _import_bass()
    if "nc" not in _CACHE:
        _CACHE["nc"] = _build_program()
    nc = _CACHE["nc"]
    in_maps = _host_prep(inputs)
    res = run_bass_kernel_spmd(nc, in_maps, list(range(NC))).results
    out = np.zeros((B, T, C), np.float32)
    for c in range(NC):
        b, j = c // 4, c % 4
        out[b, j * STRIP:(j + 1) * STRIP, :] = res[c]["out"].T
    return out
